# revision 47
# baseline (speedup 1.0000x reference)
"""Trainium2 Bass kernel for nn_Aggregator (gnn_message_passing).

Math (reference):
  yes_skip  = skip_decisions with diagonal zeroed
  no_skip   = diag(skip_decisions)
  p_bt      = ip * no_skip * branch[:,0];  p_bf = ip * no_skip * branch[:,1]
  new_ip[j] = seg_sum(p_bt, true_idx)[j] + seg_sum(p_bf, false_idx)[j]
            + sum_i ip[i]*yes_skip[i,j]
  num[j,h]  = seg_sum(h*p_bt, true_idx)[j,h] + seg_sum(h*p_bf, false_idx)[j,h]
            + sum_i hssp[i,j,h]*ip[i]*yes_skip[i,j]
  new_hid   = num / (new_ip + 1e-7)

Distribution: i (source-node) axis sharded over 8 cores (128 rows each).
Each core computes partial sums over its i rows; two ReduceScatters
combine them so core c owns output rows [128c, 128c+128); host concats.

The dominant cost is streaming the [1024,1024,256] f32 (1 GiB) tensor;
per core 128 MiB at the ~358 GB/s HBM/NC limit => ~375 us roofline.
The weighted accumulation runs as fused scalar_tensor_tensor ops
(acc = tile*w_col + acc), split between the Vector and GpSimd engines.
"""
import sys

if '/opt/trn_rl_repo' not in sys.path:
    sys.path.insert(0, '/opt/trn_rl_repo')

import numpy as np

N = 1024
H = 256
NCORES = 8
S = N // NCORES        # i rows per core
P = 128                # partition tile size for the j axis

_prog_cache = {}


G = 32                 # j's per PE chunk (32-aligned for partition bases)
IBLK = 128 // G        # i's per PE block


PE_PER_PHASE = 0


def build_program(n=N, h=H, ncores=NCORES, pe_per_phase=PE_PER_PHASE,
                  row_bufs=12, ph=None, rows_bf16=False, grp=16):
    """Build the SPMD Bass program (same program for every core).

    pe_per_phase: j-tiles per phase computed on the TensorEngine via
    block-diagonal matmuls (the rest accumulate on the Vector engine).
    """
    import concourse.bass as bass  # noqa: F401  (registers engine classes)
    import concourse.bacc as bacc
    import concourse.mybir as mybir
    from concourse import tile

    f32 = mybir.dt.float32
    s = n // ncores            # i rows per core
    jt_n = n // P              # j tiles of 128
    IB = s // IBLK             # i-blocks for the PE path
    NCB = (P // G) * IB        # (chunk, block) pairs per PE j-tile
    assert n % P == 0 and s % IBLK == 0 and s <= 128

    nc = bacc.Bacc(None)

    # ---- I/O ----
    hssp = nc.declare_dram_parameter("hssp", [s, n, h], f32, isOutput=False)
    skip = nc.declare_dram_parameter("skip", [s, n], f32, isOutput=False)
    ip = nc.declare_dram_parameter("ip", [s, 1], f32, isOutput=False)
    hprop = nc.declare_dram_parameter("hprop", [s, h], f32, isOutput=False)
    pt = nc.declare_dram_parameter("pt", [s, 1], f32, isOutput=False)
    pf = nc.declare_dram_parameter("pf", [s, 1], f32, isOutput=False)
    tidx = nc.declare_dram_parameter("tidx", [s, 1], f32, isOutput=False)
    fidx = nc.declare_dram_parameter("fidx", [s, 1], f32, isOutput=False)
    gidx = nc.declare_dram_parameter("gidx", [s, 1], f32, isOutput=False)
    iota = nc.declare_dram_parameter("iota", [s, n], f32, isOutput=False)
    ident = nc.declare_dram_parameter("ident", [s, s], f32, isOutput=False)
    out_hid = nc.declare_dram_parameter("out_hid", [s, h], f32, isOutput=True)
    out_ip = nc.declare_dram_parameter("out_ip", [s, 1], f32, isOutput=True)
    ident_pe = nc.declare_dram_parameter("ident_pe", [P, P], f32,
                                         isOutput=False)

    PH = ph if ph is not None else globals()["PH"]
    half = jt_n // PH
    assert jt_n % PH == 0 and 0 <= pe_per_phase <= half
    # PE j-tiles: the last pe_per_phase tiles of each phase
    pe_set = {p * half + (half - 1 - q)
              for p in range(PH) for q in range(pe_per_phase)}
    pe_ins = {}
    for q, jt in enumerate(sorted(pe_set)):
        pe_ins[jt] = (
            nc.declare_dram_parameter(f"skip_gi{q}", [P, NCB], f32,
                                      isOutput=False),
            nc.declare_dram_parameter(f"mask_gi{q}", [P, NCB], f32,
                                      isOutput=False),
        )
    if pe_set:
        ip_gi = nc.declare_dram_parameter("ip_gi", [P, IB], f32,
                                          isOutput=False)
        emat = nc.declare_dram_parameter("emat", [P, G], f32, isOutput=False)

    eq = mybir.AluOpType.is_equal
    mult = mybir.AluOpType.mult
    sub = mybir.AluOpType.subtract
    add = mybir.AluOpType.add

    with tile.TileContext(nc) as tc:
        with tc.tile_pool(name="dram", bufs=1, space="DRAM") as dram, \
             tc.tile_pool(name="const", bufs=1) as const, \
             tc.tile_pool(name="acc", bufs=1) as accp, \
             tc.tile_pool(name="rows", bufs=row_bufs) as rows, \
             tc.tile_pool(name="psum_t", bufs=2, space="PSUM") as psum_t, \
             tc.tile_pool(name="psum_ip", bufs=2, space="PSUM") as psum_ipp, \
             tc.tile_pool(name="psum_h", bufs=2, space="PSUM") as psum_hp:

            # collective buffers (internal DRAM); cols 0..h-1 = hidden
            # partials, col h = ip partial, cols h+1.. pad rows to 32B.
            # Two phases (j halves) so phase 1's ReduceScatter overlaps
            # phase 2's compute.
            W = h + 8
            nph = n // PH              # j rows per phase
            sph = nph // ncores        # output rows per core per phase
            hidbuf = [dram.tile([nph, W], f32, tag=f"hidbuf{p}",
                                name=f"hidbuf{p}") for p in range(PH)]
            hidscat = [dram.tile([sph, W], f32, tag=f"hidscat{p}",
                                 name=f"hidscat{p}") for p in range(PH)]

            # ---- load small inputs ----
            skip_t = const.tile([s, n], f32, tag="skip")
            iota_t = const.tile([s, n], f32, tag="iota")
            ip_t = const.tile([s, 1], f32, tag="ip")
            h_t = const.tile([s, h], f32, tag="h")
            pt_t = const.tile([s, 1], f32, tag="pt")
            pf_t = const.tile([s, 1], f32, tag="pf")
            tidx_t = const.tile([s, 1], f32, tag="tidx")
            fidx_t = const.tile([s, 1], f32, tag="fidx")
            gidx_t = const.tile([s, 1], f32, tag="gidx")
            ident_t = const.tile([s, s], f32, tag="ident")
            nc.sync.dma_start(skip_t[:], skip[:])
            nc.sync.dma_start(iota_t[:], iota[:])
            nc.sync.dma_start(ip_t[:], ip[:])
            nc.sync.dma_start(h_t[:], hprop[:])
            nc.sync.dma_start(pt_t[:], pt[:])
            nc.sync.dma_start(pf_t[:], pf[:])
            nc.sync.dma_start(tidx_t[:], tidx[:])
            nc.sync.dma_start(fidx_t[:], fidx[:])
            nc.sync.dma_start(gidx_t[:], gidx[:])
            nc.sync.dma_start(ident_t[:], ident[:])
            identpe_t = const.tile([P, P], f32, tag="identpe")
            nc.sync.dma_start(identpe_t[:], ident_pe[:])

            # ---- stage A: small tensors ----
            # diagonal one-hot D, no_skip, w = ip * yes_skip
            d_t = const.tile([s, n], f32, tag="d")
            scr_t = const.tile([s, n], f32, tag="scr")
            w_t = const.tile([s, n], f32, tag="w")
            nsk_t = const.tile([s, 1], f32, tag="nsk")
            nc.vector.tensor_scalar(d_t[:], iota_t[:], gidx_t[:], None, op0=eq)
            nc.vector.tensor_tensor(scr_t[:], skip_t[:], d_t[:], mult)
            nc.vector.reduce_sum(nsk_t[:], scr_t[:], mybir.AxisListType.X)
            nc.vector.tensor_tensor(w_t[:], skip_t[:], scr_t[:], sub)
            nc.vector.tensor_scalar_mul(w_t[:], w_t[:], ip_t[:])

            # p_branch_true/false (per-partition scalars)
            pbt_t = const.tile([s, 1], f32, tag="pbt")
            pbf_t = const.tile([s, 1], f32, tag="pbf")
            tmp1_t = const.tile([s, 1], f32, tag="tmp1")
            nc.vector.tensor_tensor(tmp1_t[:], ip_t[:], nsk_t[:], mult)
            nc.vector.tensor_tensor(pbt_t[:], tmp1_t[:], pt_t[:], mult)
            nc.vector.tensor_tensor(pbf_t[:], tmp1_t[:], pf_t[:], mult)

            # one-hot segment matrices
            tmat_t = const.tile([s, n], f32, tag="tmat")
            fmat_t = const.tile([s, n], f32, tag="fmat")
            nc.vector.tensor_scalar(tmat_t[:], iota_t[:], tidx_t[:], None, op0=eq)
            nc.vector.tensor_scalar(fmat_t[:], iota_t[:], fidx_t[:], None, op0=eq)

            # weighted hidden proposals
            pht_t = const.tile([s, h], f32, tag="pht")
            phf_t = const.tile([s, h], f32, tag="phf")
            nc.vector.tensor_scalar_mul(pht_t[:], h_t[:], pbt_t[:])
            nc.vector.tensor_scalar_mul(phf_t[:], h_t[:], pbf_t[:])

            # ones for the skip_contrib column-sum matmul
            ones_t = const.tile([s, 1], f32, tag="ones")
            nc.vector.memset(ones_t[:], 1.0)

            # wT tiles: [128 j, s i] per Vector j-tile, via PE transpose
            wT = {}
            for jt in range(jt_n):
                if jt in pe_set:
                    continue
                pt_ps = psum_t.tile([P, s], f32)
                nc.tensor.transpose(pt_ps[:], w_t[:, jt * P:(jt + 1) * P], ident_t[:])
                wt_t = accp.tile([P, s], f32, tag=f"wT{jt}",
                                 name=f"wT{jt}")
                nc.vector.tensor_copy(wt_t[:], pt_ps[:])
                wT[jt] = wt_t

            # PE-path weights: per PE j-tile, block-diag lhsT tiles
            # W_all[b][:, c*G:(c+1)*G] = E * ip_gi[:,b] * w_pe[:, c*IB+b]
            pe_W = {}
            if pe_set:
                ipgi_t = const.tile([P, IB], f32, tag="ipgi")
                emat_t = const.tile([P, G], f32, tag="emat")
                nc.sync.dma_start(ipgi_t[:], ip_gi[:])
                nc.sync.dma_start(emat_t[:], emat[:])
                eip = []
                for b in range(IB):
                    eb = const.tile([P, G], f32, tag=f"eip{b}")
                    nc.vector.tensor_scalar_mul(eb[:], emat_t[:],
                                                ipgi_t[:, b:b + 1])
                    eip.append(eb)
                for q, jt in enumerate(sorted(pe_set)):
                    sgi_t = const.tile([P, NCB], f32, tag=f"sgi{jt}")
                    mgi_t = const.tile([P, NCB], f32, tag=f"mgi{jt}")
                    nc.sync.dma_start(sgi_t[:], pe_ins[jt][0][:])
                    nc.sync.dma_start(mgi_t[:], pe_ins[jt][1][:])
                    wpe_t = const.tile([P, NCB], f32, tag=f"wpe{jt}")
                    nc.vector.tensor_tensor(wpe_t[:], sgi_t[:], mgi_t[:], mult)
                    wbs = []
                    for b in range(IB):
                        wb = const.tile([P, (P // G) * G], f32,
                                        tag=f"wall{jt}_{b}")
                        out_v = wb[:].rearrange("p (c g) -> p c g", g=G)
                        eip_v = eip[b][:].unsqueeze(1).broadcast_to(
                            (P, P // G, G))
                        wpe_v = wpe_t[:, b::IB].unsqueeze(2).broadcast_to(
                            (P, P // G, G))
                        nc.vector.tensor_tensor(out_v, wpe_v, eip_v, mult)
                        wbs.append(wb)
                    pe_W[jt] = wbs

            # ip partials: new_ip_part[j] = T'p_bt + F'p_bf + w'1
            ip_part = const.tile([1, n], f32, tag="ip_part")
            ck = min(512, n)
            for c0 in range(0, n, ck):
                ps = psum_ipp.tile([1, ck], f32)
                nc.tensor.matmul(ps[:], pbt_t[:], tmat_t[:, c0:c0 + ck],
                                 start=True, stop=False)
                nc.tensor.matmul(ps[:], pbf_t[:], fmat_t[:, c0:c0 + ck],
                                 start=False, stop=False)
                nc.tensor.matmul(ps[:], ones_t[:], w_t[:, c0:c0 + ck],
                                 start=False, stop=True)
                nc.scalar.copy(ip_part[:, c0:c0 + ck], ps[:])
            for p in range(PH):
                nc.sync.dma_start(hidbuf[p][:, h:h + 1],
                                  ip_part[:, p * nph:(p + 1) * nph])

            # ---- stage B + C: per phase, stream the hssp j-half; fused
            # scalar_tensor_tensor accumulation on the Vector engine into
            # zero-initialized SBUF accumulators.  The one-hot seg-sum
            # partials (PE matmuls into PSUM) are added at the end, off the
            # startup critical path.  Then ReduceScatter, divide, output.
            zpad_t = const.tile([P, W - h - 1], f32, tag="zpad")
            nc.vector.memset(zpad_t[:], 0.0)
            hssp_v = hssp.rearrange("i (f t p) h -> f i p t h", f=PH, p=P)
            rdt = mybir.dt.bfloat16 if rows_bf16 else f32
            for p in range(PH):
                acc = []
                accb = []
                for t in range(half):
                    a_t = accp.tile([P, h], f32, tag=f"acc{t}",
                                    name=f"acc{p}_{t}")
                    nc.vector.memset(a_t[:], 0.0)
                    acc.append(a_t)
                    if rows_bf16:
                        ab_t = accp.tile([P, h], rdt, tag=f"accb{t}",
                                         name=f"accb{p}_{t}")
                        accb.append(ab_t)
                for i in range(s):
                    if rows_bf16:
                        # SWDGE cast-DMA (f32 -> bf16); only gpsimd can cast
                        eng = nc.gpsimd
                        tag = "row"
                    else:
                        # spread DMA-descriptor generation across three DGEs
                        eng = (nc.sync, nc.scalar, nc.gpsimd)[i % 3]
                        tag = f"row{i % 3}"
                    row = rows.tile([P, half * h], rdt, tag=tag,
                                    bufs=row_bufs if rows_bf16 else 5)
                    row_v = row[:].rearrange("p (t h) -> p t h", h=h)
                    eng.dma_start(row_v, hssp_v[p, i])
                    for t in range(half):
                        jt = p * half + t
                        if not rows_bf16:
                            nc.vector.scalar_tensor_tensor(
                                acc[t][:],
                                row[:, t * h:(t + 1) * h],
                                wT[jt][:, i:i + 1],
                                acc[t][:],
                                op0=mult,
                                op1=add,
                            )
                        elif i % grp == 0:
                            # first of group: overwrite the bf16 accumulator
                            nc.vector.tensor_scalar(
                                accb[t][:], row[:, t * h:(t + 1) * h],
                                wT[jt][:, i:i + 1], None, op0=mult)
                        else:
                            nc.vector.scalar_tensor_tensor(
                                accb[t][:],
                                row[:, t * h:(t + 1) * h],
                                wT[jt][:, i:i + 1],
                                accb[t][:],
                                op0=mult,
                                op1=add,
                            )
                        if rows_bf16 and (i % grp == grp - 1 or i == s - 1):
                            # flush the bf16 group sum into the f32 acc
                            nc.vector.tensor_tensor(
                                acc[t][:], acc[t][:], accb[t][:], add)
                for t in range(half):
                    jt = p * half + t
                    ps = psum_hp.tile([P, h], f32)
                    nc.tensor.matmul(ps[:], tmat_t[:, jt * P:(jt + 1) * P],
                                     pht_t[:], start=True, stop=False)
                    nc.tensor.matmul(ps[:], fmat_t[:, jt * P:(jt + 1) * P],
                                     phf_t[:], start=False, stop=True)
                    nc.vector.tensor_tensor(acc[t][:], acc[t][:], ps[:], add)
                    nc.sync.dma_start(hidbuf[p][t * P:(t + 1) * P, :h],
                                      acc[t][:])
                    nc.sync.dma_start(hidbuf[p][t * P:(t + 1) * P, h + 1:W],
                                      zpad_t[:])
                nc.gpsimd.collective_compute(
                    "ReduceScatter", add,
                    ins=[hidbuf[p].opt()], outs=[hidscat[p].opt()],
                    replica_groups=[list(range(ncores))],
                )
                hs_t = const.tile([sph, W], f32, tag=f"hs{p}")
                den_t = const.tile([sph, 1], f32, tag=f"den{p}")
                rec_t = const.tile([sph, 1], f32, tag=f"rec{p}")
                nc.sync.dma_start(hs_t[:], hidscat[p][:])
                nc.vector.tensor_scalar_add(den_t[:], hs_t[:, h:h + 1], 1e-7)
                nc.vector.reciprocal(rec_t[:], den_t[:])
                nc.sync.dma_start(out_ip[p * sph:(p + 1) * sph, :],
                                  hs_t[:, h:h + 1])
                nc.vector.tensor_scalar_mul(hs_t[:, :h], hs_t[:, :h], rec_t[:])
                nc.sync.dma_start(out_hid[p * sph:(p + 1) * sph, :],
                                  hs_t[:, :h])

    nc.finalize()
    return nc


def make_in_maps(inputs, n=N, ncores=NCORES, pe_per_phase=PE_PER_PHASE):
    """Shard the full inputs into per-core input maps."""
    s = n // ncores
    jt_n = n // P
    half = jt_n // PH
    IB = s // IBLK
    pe_set = sorted({p * half + (half - 1 - q)
                     for p in range(PH) for q in range(pe_per_phase)})
    ipf = np.asarray(inputs["instruction_pointer"], dtype=np.float32)
    hp = np.asarray(inputs["hidden_state_proposals"], dtype=np.float32)
    hssp = np.asarray(inputs["hidden_state_skip_proposals"], dtype=np.float32)
    sk = np.asarray(inputs["skip_decisions"], dtype=np.float32)
    br = np.asarray(inputs["branch_decisions"], dtype=np.float32)
    ti = np.asarray(inputs["true_indexes"])
    fi = np.asarray(inputs["false_indexes"])

    iota = np.broadcast_to(np.arange(n, dtype=np.float32), (s, n))
    iota = np.ascontiguousarray(iota)
    ident = np.eye(s, dtype=np.float32)

    emat = np.repeat(np.eye(G, dtype=np.float32), IBLK, axis=0)  # [128, G]

    maps = []
    for c in range(ncores):
        lo, hi = c * s, (c + 1) * s
        m = {
            "hssp": hssp[lo:hi],
            "skip": sk[lo:hi],
            "ip": ipf[lo:hi].reshape(s, 1),
            "hprop": hp[lo:hi],
            "pt": np.ascontiguousarray(br[lo:hi, 0:1]),
            "pf": np.ascontiguousarray(br[lo:hi, 1:2]),
            "tidx": ti[lo:hi].astype(np.float32).reshape(s, 1),
            "fidx": fi[lo:hi].astype(np.float32).reshape(s, 1),
            "gidx": np.arange(lo, hi, dtype=np.float32).reshape(s, 1),
            "iota": iota,
            "ident": ident,
            "ident_pe": np.eye(P, dtype=np.float32),
        }
        if pe_set:
            # (g,i)-partition layouts for the PE block-diag path
            m["ip_gi"] = np.ascontiguousarray(
                np.tile(ipf[lo:hi].reshape(IB, IBLK).T, (G, 1)))
            m["emat"] = emat
            for q, jt in enumerate(pe_set):
                j0 = jt * P
                blk = sk[lo:hi, j0:j0 + P]
                sgi = blk.reshape(IB, IBLK, P // G, G).transpose(3, 1, 2, 0)
                m[f"skip_gi{q}"] = np.ascontiguousarray(
                    sgi.reshape(P, (P // G) * IB))
                jg = (j0 + G * np.arange(P // G)[None, :, None]
                      + np.arange(G)[:, None, None])          # [g, c, 1]
                ig = (lo + IBLK * np.arange(IB)[None, None, :]
                      + np.arange(IBLK)[:, None, None])        # [i, 1, b]
                mask = (jg[:, None, :, :] !=
                        ig[None, :, :, :]).astype(np.float32)
                # dims (g, i, c, b) -> [P, NCB]
                m[f"mask_gi{q}"] = np.ascontiguousarray(
                    mask.reshape(P, (P // G) * IB))
        maps.append(m)
    return maps


PH = 1


def gather_outputs(results, n=N, h=H, ncores=NCORES, ph=PH):
    """Reassemble full outputs from per-core phase-chunked shards.

    Phase p's ReduceScatter gives core c global rows
    [p*n/ph + c*spc, ... + spc) where spc = n/(ph*ncores); each core's
    out_hid stacks its ph chunks.
    """
    nph = n // ph
    spc = nph // ncores
    new_ip = np.empty(n, np.float32)
    new_hid = np.empty((n, h), np.float32)
    for c in range(ncores):
        oh = np.asarray(results[c]["out_hid"])
        oi = np.asarray(results[c]["out_ip"]).reshape(-1)
        for p in range(ph):
            g0 = p * nph + c * spc
            new_hid[g0:g0 + spc] = oh[p * spc:(p + 1) * spc]
            new_ip[g0:g0 + spc] = oi[p * spc:(p + 1) * spc]
    return new_ip, new_hid


def kernel(**inputs):
    from concourse.bass_utils import run_bass_kernel_spmd

    key = "full"
    if key not in _prog_cache:
        _prog_cache[key] = build_program()
    nc = _prog_cache[key]

    in_maps = make_in_maps(inputs)
    res = run_bass_kernel_spmd(nc, in_maps, list(range(NCORES)))
    return gather_outputs(res.results)


# revision 53
# speedup vs baseline: 1.0245x; 1.0245x over previous
"""Trainium2 Bass kernel for nn_Aggregator (gnn_message_passing).

Math (reference):
  yes_skip  = skip_decisions with diagonal zeroed
  no_skip   = diag(skip_decisions)
  p_bt      = ip * no_skip * branch[:,0];  p_bf = ip * no_skip * branch[:,1]
  new_ip[j] = seg_sum(p_bt, true_idx)[j] + seg_sum(p_bf, false_idx)[j]
            + sum_i ip[i]*yes_skip[i,j]
  num[j,h]  = seg_sum(h*p_bt, true_idx)[j,h] + seg_sum(h*p_bf, false_idx)[j,h]
            + sum_i hssp[i,j,h]*ip[i]*yes_skip[i,j]
  new_hid   = num / (new_ip + 1e-7)

Distribution: i (source-node) axis sharded over 8 cores (128 rows each).
Each core computes partial sums over its i rows; two ReduceScatters
combine them so core c owns output rows [128c, 128c+128); host concats.

The dominant cost is streaming the [1024,1024,256] f32 (1 GiB) tensor;
per core 128 MiB.  Row DMAs (1 MiB each) are issued round-robin from the
three descriptor generators (Sync-HWDGE, ACT-HWDGE, GpSimd-SWDGE) since a
single DGE caps at ~330 GB/s of 1KB-line descriptor generation.  The
weighted accumulation runs as fused scalar_tensor_tensor ops on the
Vector engine (acc = row_tile * wT_col + acc, ~400 ns per [128,256] op);
one-hot segment-sum matmuls (PE) are added at the end.  Measured
~584 us on 8 cores (HBM stream + DVE are co-bound at ~440 us, plus
~40 us entry skew/startup and ~70 us ReduceScatter + divide tail).
"""
import sys

if '/opt/trn_rl_repo' not in sys.path:
    sys.path.insert(0, '/opt/trn_rl_repo')

import numpy as np

N = 1024
H = 256
NCORES = 8
S = N // NCORES        # i rows per core
P = 128                # partition tile size for the j axis

_prog_cache = {}


G = 32                 # j's per PE chunk (32-aligned for partition bases)
IBLK = 128 // G        # i's per PE block


PE_PER_PHASE = 0


def build_program(n=N, h=H, ncores=NCORES, pe_per_phase=PE_PER_PHASE,
                  row_bufs=12, ph=None, rows_bf16=False, grp=16):
    """Build the SPMD Bass program (same program for every core).

    pe_per_phase: j-tiles per phase computed on the TensorEngine via
    block-diagonal matmuls (the rest accumulate on the Vector engine).
    """
    import concourse.bass as bass  # noqa: F401  (registers engine classes)
    import concourse.bacc as bacc
    import concourse.mybir as mybir
    from concourse import tile

    f32 = mybir.dt.float32
    s = n // ncores            # i rows per core
    jt_n = n // P              # j tiles of 128
    IB = s // IBLK             # i-blocks for the PE path
    NCB = (P // G) * IB        # (chunk, block) pairs per PE j-tile
    assert n % P == 0 and s % IBLK == 0 and s <= 128

    nc = bacc.Bacc(None)

    # ---- I/O ----
    hssp = nc.declare_dram_parameter("hssp", [s, n, h], f32, isOutput=False)
    skip = nc.declare_dram_parameter("skip", [s, n], f32, isOutput=False)
    ip = nc.declare_dram_parameter("ip", [s, 1], f32, isOutput=False)
    hprop = nc.declare_dram_parameter("hprop", [s, h], f32, isOutput=False)
    pt = nc.declare_dram_parameter("pt", [s, 1], f32, isOutput=False)
    pf = nc.declare_dram_parameter("pf", [s, 1], f32, isOutput=False)
    tidx = nc.declare_dram_parameter("tidx", [s, 1], f32, isOutput=False)
    fidx = nc.declare_dram_parameter("fidx", [s, 1], f32, isOutput=False)
    gidx = nc.declare_dram_parameter("gidx", [s, 1], f32, isOutput=False)
    iota = nc.declare_dram_parameter("iota", [s, n], f32, isOutput=False)
    ident = nc.declare_dram_parameter("ident", [s, s], f32, isOutput=False)
    out_hid = nc.declare_dram_parameter("out_hid", [s, h], f32, isOutput=True)
    out_ip = nc.declare_dram_parameter("out_ip", [s, 1], f32, isOutput=True)
    ident_pe = nc.declare_dram_parameter("ident_pe", [P, P], f32,
                                         isOutput=False)

    PH = ph if ph is not None else globals()["PH"]
    half = jt_n // PH
    assert jt_n % PH == 0 and 0 <= pe_per_phase <= half
    # PE j-tiles: the last pe_per_phase tiles of each phase
    pe_set = {p * half + (half - 1 - q)
              for p in range(PH) for q in range(pe_per_phase)}
    pe_ins = {}
    for q, jt in enumerate(sorted(pe_set)):
        pe_ins[jt] = (
            nc.declare_dram_parameter(f"skip_gi{q}", [P, NCB], f32,
                                      isOutput=False),
            nc.declare_dram_parameter(f"mask_gi{q}", [P, NCB], f32,
                                      isOutput=False),
        )
    if pe_set:
        ip_gi = nc.declare_dram_parameter("ip_gi", [P, IB], f32,
                                          isOutput=False)
        emat = nc.declare_dram_parameter("emat", [P, G], f32, isOutput=False)

    eq = mybir.AluOpType.is_equal
    mult = mybir.AluOpType.mult
    sub = mybir.AluOpType.subtract
    add = mybir.AluOpType.add

    with tile.TileContext(nc) as tc:
        with tc.tile_pool(name="dram", bufs=1, space="DRAM") as dram, \
             tc.tile_pool(name="const", bufs=1) as const, \
             tc.tile_pool(name="acc", bufs=1) as accp, \
             tc.tile_pool(name="rows", bufs=row_bufs) as rows, \
             tc.tile_pool(name="psum_t", bufs=2, space="PSUM") as psum_t, \
             tc.tile_pool(name="psum_ip", bufs=2, space="PSUM") as psum_ipp, \
             tc.tile_pool(name="psum_h", bufs=2, space="PSUM") as psum_hp:

            # collective buffers (internal DRAM); cols 0..h-1 = hidden
            # partials, col h = ip partial, cols h+1.. pad rows to 32B.
            # Two phases (j halves) so phase 1's ReduceScatter overlaps
            # phase 2's compute.
            W = h + 8
            nph = n // PH              # j rows per phase
            sph = nph // ncores        # output rows per core per phase
            hidbuf = [dram.tile([nph, W], f32, tag=f"hidbuf{p}",
                                name=f"hidbuf{p}") for p in range(PH)]
            hidscat = [dram.tile([sph, W], f32, tag=f"hidscat{p}",
                                 name=f"hidscat{p}") for p in range(PH)]

            # ---- load small inputs ----
            skip_t = const.tile([s, n], f32, tag="skip")
            iota_t = const.tile([s, n], f32, tag="iota")
            ip_t = const.tile([s, 1], f32, tag="ip")
            h_t = const.tile([s, h], f32, tag="h")
            pt_t = const.tile([s, 1], f32, tag="pt")
            pf_t = const.tile([s, 1], f32, tag="pf")
            tidx_t = const.tile([s, 1], f32, tag="tidx")
            fidx_t = const.tile([s, 1], f32, tag="fidx")
            gidx_t = const.tile([s, 1], f32, tag="gidx")
            ident_t = const.tile([s, s], f32, tag="ident")
            nc.sync.dma_start(skip_t[:], skip[:])
            nc.sync.dma_start(iota_t[:], iota[:])
            nc.sync.dma_start(ip_t[:], ip[:])
            nc.sync.dma_start(h_t[:], hprop[:])
            nc.sync.dma_start(pt_t[:], pt[:])
            nc.sync.dma_start(pf_t[:], pf[:])
            nc.sync.dma_start(tidx_t[:], tidx[:])
            nc.sync.dma_start(fidx_t[:], fidx[:])
            nc.sync.dma_start(gidx_t[:], gidx[:])
            nc.sync.dma_start(ident_t[:], ident[:])
            identpe_t = const.tile([P, P], f32, tag="identpe")
            nc.sync.dma_start(identpe_t[:], ident_pe[:])

            # ---- stage A: small tensors ----
            # diagonal one-hot D, no_skip, w = ip * yes_skip
            d_t = const.tile([s, n], f32, tag="d")
            scr_t = const.tile([s, n], f32, tag="scr")
            w_t = const.tile([s, n], f32, tag="w")
            nsk_t = const.tile([s, 1], f32, tag="nsk")
            nc.vector.tensor_scalar(d_t[:], iota_t[:], gidx_t[:], None, op0=eq)
            nc.vector.tensor_tensor(scr_t[:], skip_t[:], d_t[:], mult)
            nc.vector.reduce_sum(nsk_t[:], scr_t[:], mybir.AxisListType.X)
            nc.vector.tensor_tensor(w_t[:], skip_t[:], scr_t[:], sub)
            nc.vector.tensor_scalar_mul(w_t[:], w_t[:], ip_t[:])

            # p_branch_true/false (per-partition scalars)
            pbt_t = const.tile([s, 1], f32, tag="pbt")
            pbf_t = const.tile([s, 1], f32, tag="pbf")
            tmp1_t = const.tile([s, 1], f32, tag="tmp1")
            nc.vector.tensor_tensor(tmp1_t[:], ip_t[:], nsk_t[:], mult)
            nc.vector.tensor_tensor(pbt_t[:], tmp1_t[:], pt_t[:], mult)
            nc.vector.tensor_tensor(pbf_t[:], tmp1_t[:], pf_t[:], mult)

            # one-hot segment matrices
            tmat_t = const.tile([s, n], f32, tag="tmat")
            fmat_t = const.tile([s, n], f32, tag="fmat")
            nc.vector.tensor_scalar(tmat_t[:], iota_t[:], tidx_t[:], None, op0=eq)
            nc.vector.tensor_scalar(fmat_t[:], iota_t[:], fidx_t[:], None, op0=eq)

            # weighted hidden proposals
            pht_t = const.tile([s, h], f32, tag="pht")
            phf_t = const.tile([s, h], f32, tag="phf")
            nc.vector.tensor_scalar_mul(pht_t[:], h_t[:], pbt_t[:])
            nc.vector.tensor_scalar_mul(phf_t[:], h_t[:], pbf_t[:])

            # ones for the skip_contrib column-sum matmul
            ones_t = const.tile([s, 1], f32, tag="ones")
            nc.vector.memset(ones_t[:], 1.0)

            # wT tiles: [128 j, s i] per Vector j-tile, via PE transpose
            wT = {}
            for jt in range(jt_n):
                if jt in pe_set:
                    continue
                pt_ps = psum_t.tile([P, s], f32)
                nc.tensor.transpose(pt_ps[:], w_t[:, jt * P:(jt + 1) * P], ident_t[:])
                wt_t = accp.tile([P, s], f32, tag=f"wT{jt}",
                                 name=f"wT{jt}")
                nc.vector.tensor_copy(wt_t[:], pt_ps[:])
                wT[jt] = wt_t

            # PE-path weights: per PE j-tile, block-diag lhsT tiles
            # W_all[b][:, c*G:(c+1)*G] = E * ip_gi[:,b] * w_pe[:, c*IB+b]
            pe_W = {}
            if pe_set:
                ipgi_t = const.tile([P, IB], f32, tag="ipgi")
                emat_t = const.tile([P, G], f32, tag="emat")
                nc.sync.dma_start(ipgi_t[:], ip_gi[:])
                nc.sync.dma_start(emat_t[:], emat[:])
                eip = []
                for b in range(IB):
                    eb = const.tile([P, G], f32, tag=f"eip{b}")
                    nc.vector.tensor_scalar_mul(eb[:], emat_t[:],
                                                ipgi_t[:, b:b + 1])
                    eip.append(eb)
                for q, jt in enumerate(sorted(pe_set)):
                    sgi_t = const.tile([P, NCB], f32, tag=f"sgi{jt}")
                    mgi_t = const.tile([P, NCB], f32, tag=f"mgi{jt}")
                    nc.sync.dma_start(sgi_t[:], pe_ins[jt][0][:])
                    nc.sync.dma_start(mgi_t[:], pe_ins[jt][1][:])
                    wpe_t = const.tile([P, NCB], f32, tag=f"wpe{jt}")
                    nc.vector.tensor_tensor(wpe_t[:], sgi_t[:], mgi_t[:], mult)
                    wbs = []
                    for b in range(IB):
                        wb = const.tile([P, (P // G) * G], f32,
                                        tag=f"wall{jt}_{b}")
                        out_v = wb[:].rearrange("p (c g) -> p c g", g=G)
                        eip_v = eip[b][:].unsqueeze(1).broadcast_to(
                            (P, P // G, G))
                        wpe_v = wpe_t[:, b::IB].unsqueeze(2).broadcast_to(
                            (P, P // G, G))
                        nc.vector.tensor_tensor(out_v, wpe_v, eip_v, mult)
                        wbs.append(wb)
                    pe_W[jt] = wbs

            # ip partials: new_ip_part[j] = T'p_bt + F'p_bf + w'1
            ip_part = const.tile([1, n], f32, tag="ip_part")
            ck = min(512, n)
            for c0 in range(0, n, ck):
                ps = psum_ipp.tile([1, ck], f32)
                nc.tensor.matmul(ps[:], pbt_t[:], tmat_t[:, c0:c0 + ck],
                                 start=True, stop=False)
                nc.tensor.matmul(ps[:], pbf_t[:], fmat_t[:, c0:c0 + ck],
                                 start=False, stop=False)
                nc.tensor.matmul(ps[:], ones_t[:], w_t[:, c0:c0 + ck],
                                 start=False, stop=True)
                nc.scalar.copy(ip_part[:, c0:c0 + ck], ps[:])
            for p in range(PH):
                nc.sync.dma_start(hidbuf[p][:, h:h + 1],
                                  ip_part[:, p * nph:(p + 1) * nph])

            # ---- stage B + C: per phase, stream the hssp j-half; fused
            # scalar_tensor_tensor accumulation on the Vector engine into
            # zero-initialized SBUF accumulators.  The one-hot seg-sum
            # partials (PE matmuls into PSUM) are added at the end, off the
            # startup critical path.  Then ReduceScatter, divide, output.
            zpad_t = const.tile([P, W - h - 1], f32, tag="zpad")
            nc.vector.memset(zpad_t[:], 0.0)
            hssp_v = hssp.rearrange("i (f t p) h -> f i p t h", f=PH, p=P)
            rdt = mybir.dt.bfloat16 if rows_bf16 else f32
            for p in range(PH):
                acc = []
                accb = []
                for t in range(half):
                    a_t = accp.tile([P, h], f32, tag=f"acc{t}",
                                    name=f"acc{p}_{t}")
                    nc.vector.memset(a_t[:], 0.0)
                    acc.append(a_t)
                    if rows_bf16:
                        ab_t = accp.tile([P, h], rdt, tag=f"accb{t}",
                                         name=f"accb{p}_{t}")
                        accb.append(ab_t)
                for i in range(s):
                    if rows_bf16:
                        # SWDGE cast-DMA (f32 -> bf16); only gpsimd can cast
                        eng = nc.gpsimd
                        tag = "row"
                    else:
                        # spread DMA-descriptor generation across three DGEs
                        eng = (nc.sync, nc.scalar, nc.gpsimd)[i % 3]
                        tag = f"row{i % 3}"
                    row = rows.tile([P, half * h], rdt, tag=tag,
                                    bufs=row_bufs if rows_bf16 else 4)
                    row_v = row[:].rearrange("p (t h) -> p t h", h=h)
                    eng.dma_start(row_v, hssp_v[p, i])
                    for t in range(half):
                        jt = p * half + t
                        if not rows_bf16:
                            nc.vector.scalar_tensor_tensor(
                                acc[t][:],
                                row[:, t * h:(t + 1) * h],
                                wT[jt][:, i:i + 1],
                                acc[t][:],
                                op0=mult,
                                op1=add,
                            )
                        elif i % grp == 0:
                            # first of group: overwrite the bf16 accumulator
                            nc.vector.tensor_scalar(
                                accb[t][:], row[:, t * h:(t + 1) * h],
                                wT[jt][:, i:i + 1], None, op0=mult)
                        else:
                            nc.vector.scalar_tensor_tensor(
                                accb[t][:],
                                row[:, t * h:(t + 1) * h],
                                wT[jt][:, i:i + 1],
                                accb[t][:],
                                op0=mult,
                                op1=add,
                            )
                        if rows_bf16 and (i % grp == grp - 1 or i == s - 1):
                            # flush the bf16 group sum into the f32 acc
                            nc.vector.tensor_tensor(
                                acc[t][:], acc[t][:], accb[t][:], add)
                for t in range(half):
                    jt = p * half + t
                    ps = psum_hp.tile([P, h], f32)
                    nc.tensor.matmul(ps[:], tmat_t[:, jt * P:(jt + 1) * P],
                                     pht_t[:], start=True, stop=False)
                    nc.tensor.matmul(ps[:], fmat_t[:, jt * P:(jt + 1) * P],
                                     phf_t[:], start=False, stop=True)
                    nc.vector.tensor_tensor(acc[t][:], acc[t][:], ps[:], add)
                    nc.sync.dma_start(hidbuf[p][t * P:(t + 1) * P, :h],
                                      acc[t][:])
                    nc.sync.dma_start(hidbuf[p][t * P:(t + 1) * P, h + 1:W],
                                      zpad_t[:])
                nc.gpsimd.collective_compute(
                    "ReduceScatter", add,
                    ins=[hidbuf[p].opt()], outs=[hidscat[p].opt()],
                    replica_groups=[list(range(ncores))],
                )
                hs_t = const.tile([sph, W], f32, tag=f"hs{p}")
                den_t = const.tile([sph, 1], f32, tag=f"den{p}")
                rec_t = const.tile([sph, 1], f32, tag=f"rec{p}")
                nc.sync.dma_start(hs_t[:], hidscat[p][:])
                nc.vector.tensor_scalar_add(den_t[:], hs_t[:, h:h + 1], 1e-7)
                nc.vector.reciprocal(rec_t[:], den_t[:])
                nc.sync.dma_start(out_ip[p * sph:(p + 1) * sph, :],
                                  hs_t[:, h:h + 1])
                nc.vector.tensor_scalar_mul(hs_t[:, :h], hs_t[:, :h], rec_t[:])
                nc.sync.dma_start(out_hid[p * sph:(p + 1) * sph, :],
                                  hs_t[:, :h])

    nc.finalize()
    return nc


def make_in_maps(inputs, n=N, ncores=NCORES, pe_per_phase=PE_PER_PHASE):
    """Shard the full inputs into per-core input maps."""
    s = n // ncores
    jt_n = n // P
    half = jt_n // PH
    IB = s // IBLK
    pe_set = sorted({p * half + (half - 1 - q)
                     for p in range(PH) for q in range(pe_per_phase)})
    ipf = np.asarray(inputs["instruction_pointer"], dtype=np.float32)
    hp = np.asarray(inputs["hidden_state_proposals"], dtype=np.float32)
    hssp = np.asarray(inputs["hidden_state_skip_proposals"], dtype=np.float32)
    sk = np.asarray(inputs["skip_decisions"], dtype=np.float32)
    br = np.asarray(inputs["branch_decisions"], dtype=np.float32)
    ti = np.asarray(inputs["true_indexes"])
    fi = np.asarray(inputs["false_indexes"])

    iota = np.broadcast_to(np.arange(n, dtype=np.float32), (s, n))
    iota = np.ascontiguousarray(iota)
    ident = np.eye(s, dtype=np.float32)

    emat = np.repeat(np.eye(G, dtype=np.float32), IBLK, axis=0)  # [128, G]

    maps = []
    for c in range(ncores):
        lo, hi = c * s, (c + 1) * s
        m = {
            "hssp": hssp[lo:hi],
            "skip": sk[lo:hi],
            "ip": ipf[lo:hi].reshape(s, 1),
            "hprop": hp[lo:hi],
            "pt": np.ascontiguousarray(br[lo:hi, 0:1]),
            "pf": np.ascontiguousarray(br[lo:hi, 1:2]),
            "tidx": ti[lo:hi].astype(np.float32).reshape(s, 1),
            "fidx": fi[lo:hi].astype(np.float32).reshape(s, 1),
            "gidx": np.arange(lo, hi, dtype=np.float32).reshape(s, 1),
            "iota": iota,
            "ident": ident,
            "ident_pe": np.eye(P, dtype=np.float32),
        }
        if pe_set:
            # (g,i)-partition layouts for the PE block-diag path
            m["ip_gi"] = np.ascontiguousarray(
                np.tile(ipf[lo:hi].reshape(IB, IBLK).T, (G, 1)))
            m["emat"] = emat
            for q, jt in enumerate(pe_set):
                j0 = jt * P
                blk = sk[lo:hi, j0:j0 + P]
                sgi = blk.reshape(IB, IBLK, P // G, G).transpose(3, 1, 2, 0)
                m[f"skip_gi{q}"] = np.ascontiguousarray(
                    sgi.reshape(P, (P // G) * IB))
                jg = (j0 + G * np.arange(P // G)[None, :, None]
                      + np.arange(G)[:, None, None])          # [g, c, 1]
                ig = (lo + IBLK * np.arange(IB)[None, None, :]
                      + np.arange(IBLK)[:, None, None])        # [i, 1, b]
                mask = (jg[:, None, :, :] !=
                        ig[None, :, :, :]).astype(np.float32)
                # dims (g, i, c, b) -> [P, NCB]
                m[f"mask_gi{q}"] = np.ascontiguousarray(
                    mask.reshape(P, (P // G) * IB))
        maps.append(m)
    return maps


PH = 1


def gather_outputs(results, n=N, h=H, ncores=NCORES, ph=PH):
    """Reassemble full outputs from per-core phase-chunked shards.

    Phase p's ReduceScatter gives core c global rows
    [p*n/ph + c*spc, ... + spc) where spc = n/(ph*ncores); each core's
    out_hid stacks its ph chunks.
    """
    nph = n // ph
    spc = nph // ncores
    new_ip = np.empty(n, np.float32)
    new_hid = np.empty((n, h), np.float32)
    for c in range(ncores):
        oh = np.asarray(results[c]["out_hid"])
        oi = np.asarray(results[c]["out_ip"]).reshape(-1)
        for p in range(ph):
            g0 = p * nph + c * spc
            new_hid[g0:g0 + spc] = oh[p * spc:(p + 1) * spc]
            new_ip[g0:g0 + spc] = oi[p * spc:(p + 1) * spc]
    return new_ip, new_hid


def kernel(**inputs):
    from concourse.bass_utils import run_bass_kernel_spmd

    key = "full"
    if key not in _prog_cache:
        _prog_cache[key] = build_program()
    nc = _prog_cache[key]

    in_maps = make_in_maps(inputs)
    res = run_bass_kernel_spmd(nc, in_maps, list(range(NCORES)))
    return gather_outputs(res.results)
